# revision 21
# baseline (speedup 1.0000x reference)
"""TRN2 Bass/Tile kernel: deformable-kernel spatial attention (dense_cnn).

v3 restructure (vs v2 baseline, 371.7us):
  - x is fed to the device as bf16 (CPU-side cast in kernel()): HBM read
    halves to 8MB and x DMAs straight into the SBUF tile (no f32 staging,
    no DVE casts).  xsb drops its column gutters so every x transfer is
    64 contiguous 4KB descriptors; the conv1 left-edge (kx=0) taps
    accumulate into ps[:, :, 1:128] instead of reading a zero gutter.
  - the scattered gather-DMAs for fc/dkc/w3 weights (~16k 4-byte
    descriptors that jammed the DMA engines mid x-load) are replaced by
    contiguous loads + batched PE transposes.
  - output is stored as bf16 (8MB) and upcast to f32 in numpy.
  - dkc: per-tap diag weight tiles (first matmul starts after the first
    build), DVE taps + diag builds read samp psum directly (no SBUF
    copy of samp).

Per-core (pure data parallel, batch 8 over 8 cores):
  h1 = relu(conv1(x))     3x3 stride-2 64->64, 9 tap-matmuls, M=128 block-diag
  5x dkc:                 global-pool -> fc offsets -> gather-free bilinear
                          resample of the 4x4 scope kernel -> depthwise 3x3
                          as diag matmuls per block (full 128 partitions)
  conv2+pixel_shuffle+conv3 fused into 64->4 channel 3x3 conv (M=8)
  att = sigmoid(logits); replicate att to 128 partitions with selector
  matmuls into PSUM; DVE multiplies x (SBUF bf16) from PSUM; DMA out bf16.
"""

import numpy as np
import ml_dtypes

import concourse.bass as bass
import concourse.mybir as mybir
import concourse.tile as tile
from concourse import bacc
from concourse.bass_utils import run_bass_kernel_spmd
from concourse.masks import make_identity
from contextlib import ExitStack

f32 = mybir.dt.float32
f32r = mybir.dt.float32r
bf16 = mybir.dt.bfloat16
i32 = mybir.dt.int32
AF = mybir.ActivationFunctionType
ALU = mybir.AluOpType
AX = mybir.AxisListType

C = 64
H = 256
HH = 128
RB = 64          # interior rows per half of h
SLOTS = RB + 2   # + top/bottom halo row
WCOL = HH + 2    # zero gutter columns at 0 and 129 (h tiles only)
NL = 5
XS = HH + 1      # x row slots per half (slot 0 = halo/zero row)

TAPS = [(t // 3 - 1, t % 3 - 1) for t in range(9)]  # t = 3*ty+tx -> (dy, dx)

# dkc tap split: PE diag-matmuls for most taps, DVE psum-FMA for the rest
DVE_TAPS = [5, 7]
PE_TAPS = [t for t in range(9) if t not in DVE_TAPS]

# conv1 tap order: kx=0 taps (t=0,3,6) read input col -1 at x=0, so they
# go last and accumulate into ps[:, :, 1:128] only (zero-pad semantics).
CONV1_ORDER = [1, 2, 4, 5, 7, 8, 0, 3, 6]


def _ap(a, extra_off, dims):
    return bass.AP(tensor=a.tensor, offset=a.offset + extra_off, ap=dims)


def build_nc():
    nc = bacc.Bacc("TRN2", target_bir_lowering=False, debug=False)
    x_d = nc.dram_tensor("x", [C, H, H], bf16, kind="ExternalInput").ap()
    w1_d = nc.dram_tensor("conv1_w", [C, C, 3, 3], f32, kind="ExternalInput").ap()
    b1_d = nc.dram_tensor("conv1_b", [C], f32, kind="ExternalInput").ap()
    dkw_d = nc.dram_tensor("dkc_w", [NL, C, 1, 4, 4], f32, kind="ExternalInput").ap()
    dkb_d = nc.dram_tensor("dkc_b", [NL, C], f32, kind="ExternalInput").ap()
    fcw_d = nc.dram_tensor("dkc_fc_w", [NL, 18, C], f32, kind="ExternalInput").ap()
    fcb_d = nc.dram_tensor("dkc_fc_b", [NL, 18], f32, kind="ExternalInput").ap()
    w2_d = nc.dram_tensor("conv2_w", [4 * C, C, 3, 3], f32, kind="ExternalInput").ap()
    b2_d = nc.dram_tensor("conv2_b", [4 * C], f32, kind="ExternalInput").ap()
    w3_d = nc.dram_tensor("conv3_w", [1, C, 1, 1], f32, kind="ExternalInput").ap()
    b3_d = nc.dram_tensor("conv3_b", [1], f32, kind="ExternalInput").ap()
    o_d = nc.dram_tensor("out", [C, H, H], bf16, kind="ExternalOutput").ap()

    with tile.TileContext(nc) as tc:
        with ExitStack() as ctx:
            _kernel(ctx, tc, nc, x_d, w1_d, b1_d, dkw_d, dkb_d, fcw_d, fcb_d,
                    w2_d, b2_d, w3_d, b3_d, o_d)
    nc.compile()
    return nc


def _kernel(ctx, tc, nc, x_d, w1_d, b1_d, dkw_d, dkb_d, fcw_d, fcb_d,
            w2_d, b2_d, w3_d, b3_d, o_d):
    persist = ctx.enter_context(tc.tile_pool(name="persist", bufs=1))
    hpool = ctx.enter_context(tc.tile_pool(name="h", bufs=2))
    small = ctx.enter_context(tc.tile_pool(name="small", bufs=4))
    diagp = ctx.enter_context(tc.tile_pool(name="diag", bufs=9))
    outp = ctx.enter_context(tc.tile_pool(name="outb", bufs=3))
    psum = ctx.enter_context(tc.tile_pool(name="psum", bufs=4, space="PSUM"))
    psmall = ctx.enter_context(tc.tile_pool(name="psmall", bufs=1, space="PSUM"))
    pattn = ctx.enter_context(tc.tile_pool(name="pattn", bufs=3, space="PSUM"))

    # ---------------- one-time setup ----------------
    zrow = persist.tile([128, H], f32)
    nc.vector.memset(zrow[:], 0.0)

    i128f = persist.tile([128, 128], f32)
    make_identity(nc, i128f[:])

    # conv1 weights: ONE contiguous DMA (64 descriptors), then 9 PE
    # transposes [cout, cin] -> [cin, cout] batched into 2 psum tiles.
    w1c2 = persist.tile([C, 576], f32)
    nc.gpsimd.dma_start(out=w1c2[:], in_=_ap(w1_d, 0, [[576, C], [1, 576]]))
    w1bd = persist.tile([128, 9, 128], bf16)
    nc.gpsimd.memset(w1bd[:], 0.0)
    for grp, t0 in ((5, 0), (4, 5)):
        wps = psmall.tile([C, grp, C], f32, tag="sp", name=f"w1ps_{t0}")
        for k in range(grp):
            t = t0 + k
            nc.tensor.transpose(wps[:, k, :], _ap(w1c2, t, [w1c2.ap[0], [9, C]]),
                                i128f[0:C, 0:C])
        nc.scalar.activation(w1bd[0:C, t0:t0 + grp, 0:C],
                             wps[:], AF.Copy, bias=0.0, scale=1.0)
    nc.gpsimd.dma_start(out=w1bd[C:128, :, C:128], in_=w1bd[0:C, :, 0:C])

    biases = persist.tile([128, 6], f32)
    nc.gpsimd.dma_start(out=biases[0:C, 0:1], in_=b1_d.unsqueeze(-1))
    for i in range(NL):
        nc.gpsimd.dma_start(out=biases[0:C, 1 + i:2 + i], in_=dkb_d[i].unsqueeze(-1))
    nc.gpsimd.dma_start(out=biases[C:128, 0:6], in_=biases[0:C, 0:6])

    # fc weights [5, 18, 64] -> load [18, 5, 64] contiguous rows, PE
    # transpose per layer -> fcwt [64, 5, 18] (f32r).
    fcw_raw = persist.tile([18, NL, C], f32)
    nc.gpsimd.dma_start(out=fcw_raw[:],
                        in_=_ap(fcw_d, 0, [[C, 18], [18 * C, NL], [1, C]]))
    fcwt = persist.tile([C, NL, 18], f32r)
    fps = psmall.tile([C, NL, 18], f32, tag="sp", name="fps_all")
    for li in range(NL):
        nc.tensor.transpose(fps[:, li, :], fcw_raw[:, li, :], i128f[0:18, 0:18])
    nc.scalar.activation(fcwt[:], fps[:], AF.Copy, bias=0.0, scale=1.0)

    # dkc scope weights [5, 64, 16] -> load [64, 5, 16] contiguous rows,
    # PE transpose per layer -> w2dt2 [16, 5, 128] (f32r, dup halves).
    dkw_raw = persist.tile([C, NL, 16], f32)
    nc.gpsimd.dma_start(out=dkw_raw[:],
                        in_=_ap(dkw_d, 0, [[16, C], [1024, NL], [1, 16]]))
    w2dt2 = persist.tile([16, NL, 128], f32r)
    wps2 = psmall.tile([16, NL, C], f32, tag="sp", name="wps_all")
    for li in range(NL):
        nc.tensor.transpose(wps2[:, li, :], dkw_raw[:, li, :], i128f[0:C, 0:C])
    nc.scalar.activation(w2dt2[:, :, 0:C], wps2[:], AF.Copy, bias=0.0, scale=1.0)
    nc.gpsimd.dma_start(out=w2dt2[:, :, C:128], in_=w2dt2[:, :, 0:C])

    # ---------------- x staged in SBUF (bf16, halo slot 0, NO col gutters) --
    # top half: slot s = x row s-1 (slot 0 = zero); bottom: slot s = row 127+s
    xsb = persist.tile([128, XS, H], bf16)
    nc.vector.memset(xsb[0:C, 0, :], 0.0)
    nc.sync.dma_start(out=xsb[C:128, 0, :], in_=x_d[:, 127:128, :])

    # ---------------- h tensors ----------------
    def new_h():
        h = hpool.tile([128, SLOTS, WCOL], bf16, tag="h")
        nc.scalar.activation(h[0:C, 0, :], zrow[0:C, 0:WCOL], AF.Copy, bias=0.0, scale=1.0)
        nc.scalar.activation(h[C:128, SLOTS - 1, :], zrow[C:128, 0:WCOL], AF.Copy, bias=0.0, scale=1.0)
        zc2 = _ap(zrow, 0, [zrow.ap[0], [1, SLOTS], [1, 1]])
        nc.scalar.activation(h[:, :, 0:1], zc2, AF.Copy, bias=0.0, scale=1.0)
        nc.scalar.activation(h[:, :, WCOL - 1:WCOL], zc2, AF.Copy, bias=0.0, scale=1.0)
        return h

    def halo_fix(h):
        nc.gpsimd.dma_start(out=h[C:128, 0, :], in_=h[0:C, RB, :])
        nc.gpsimd.dma_start(out=h[0:C, SLOTS - 1, :], in_=h[C:128, 1, :])

    # ---------------- conv1 (+ x stream-in), 1-block software pipeline ------
    # x block b DMAs straight into xsb (bf16, contiguous 4KB/partition).
    # conv1 block b needs x blocks b-1 (halo) and b.
    h1 = new_h()
    pp1 = small.tile([128, 16], f32, tag="pp")
    with nc.named_scope("conv1"):
        # bottom-half x triggers all up front on the Act queue (nothing
        # else queued there yet, so the transfers all issue within ~11us);
        # top halves on sync inside the loop.
        for b in range(16):
            nc.scalar.dma_start(out=xsb[C:128, 8 * b + 1:8 * b + 9, :],
                                in_=x_d[:, 128 + 8 * b:128 + 8 * b + 8, :])
        for b in range(17):
            if b < 16:
                nc.sync.dma_start(out=xsb[0:C, 8 * b + 1:8 * b + 9, :],
                                  in_=x_d[:, 8 * b:8 * b + 8, :])
            if b >= 1:
                bb = b - 1
                ps = psum.tile([128, 4, HH], f32, tag="cps")
                for ti, t in enumerate(CONV1_ORDER):
                    dy, dx = TAPS[t]
                    ky, kx = dy + 1, dx + 1
                    rows = xsb[:, 8 * bb + ky:8 * bb + ky + 7:2, :]
                    if kx == 0:
                        # left-edge tap: output x=0 would read input col -1;
                        # accumulate into ps[:, :, 1:] only (zero padding).
                        rhs = _ap(rows, 1, [rows.ap[0], rows.ap[1], [2, HH - 1]])
                        out_ap = ps[:, :, 1:HH]
                    else:
                        rhs = _ap(rows, kx - 1, [rows.ap[0], rows.ap[1], [2, HH]])
                        out_ap = ps[:]
                    nc.tensor.matmul(out_ap, w1bd[:, t, :], rhs,
                                     start=(ti == 0), stop=(ti == 8),
                                     skip_group_check=True)
                s0 = 1 + 4 * bb
                nc.scalar.activation(h1[:, s0:s0 + 4, 1:HH + 1], ps[:],
                                     AF.Relu, bias=biases[:, 0:1], scale=1.0,
                                     accum_out=pp1[:, bb:bb + 1])
    # ------- deferred setup: emitted after conv1 so the PE work (w2f build,
    # ------- G build) lands in the conv1->dkc0 pipeline bubble ------
    # G[p, li, m] = fc_w[li, m, p % 64] / 16384: folds the cross-half pool
    # reduction AND the fc matmul into one per-layer matmul
    # (off = G.T @ red_replicated, contraction over all 128 partitions).
    st2Tf = persist.tile([C, 128], f32)
    st2Tr = persist.tile([C, 128], f32r)
    nc.gpsimd.memset(st2Tf[:], 0.0)
    for g in range(2):
        nc.gpsimd.affine_select(out=st2Tf[:, C * g:C * g + C],
                                in_=st2Tf[:, C * g:C * g + C],
                                pattern=[[-1, C]],
                                compare_op=ALU.not_equal,
                                fill=1.0 / 16384.0, base=0, channel_multiplier=1)
    nc.scalar.activation(st2Tr[:], st2Tf[:], AF.Copy, bias=0.0, scale=1.0)
    G = persist.tile([128, NL, 18], f32r)
    for li in range(NL):
        g_ps = psmall.tile([128, 18], f32, tag="sp", name=f"g_ps_{li}")
        nc.tensor.matmul(g_ps[:], st2Tr[:], fcwt[:, li, :], start=True, stop=True)
        nc.scalar.activation(G[:, li, :], g_ps[:], AF.Copy, bias=0.0, scale=1.0)
    ones12816 = persist.tile([128, 16], f32)
    nc.vector.memset(ones12816, 1.0)
    red16 = persist.tile([128, 16], f32r)

    it = small.tile([16, 9], i32, tag="it")
    by16 = persist.tile([16, 9], f32)
    bx16 = persist.tile([16, 9], f32)
    nc.gpsimd.iota(it[:], pattern=[[1, 3], [0, 3]], base=0, channel_multiplier=0)
    nc.vector.tensor_copy(by16[:], it[:])
    nc.vector.tensor_scalar_add(by16[:], by16[:], 0.5)
    nc.gpsimd.iota(it[:], pattern=[[0, 3], [1, 3]], base=0, channel_multiplier=0)
    nc.vector.tensor_copy(bx16[:], it[:])
    nc.vector.tensor_scalar_add(bx16[:], bx16[:], 0.5)
    itp = small.tile([16, 1], i32, tag="itp")
    idx16 = small.tile([16, 1], f32, tag="idx16")
    ky16 = persist.tile([16, 1], f32)
    kx16 = persist.tile([16, 1], f32)
    nc.gpsimd.iota(itp[:], pattern=[[0, 1]], base=0, channel_multiplier=1)
    nc.vector.tensor_copy(idx16[:], itp[:])
    st16 = small.tile([1, 16], i32, tag="st16")
    nc.gpsimd.iota(st16[:], pattern=[[1, 4], [0, 4]], base=0, channel_multiplier=0)
    stf = small.tile([1, 16], f32, tag="stf")
    nc.vector.tensor_copy(stf[:], st16[:])
    nc.gpsimd.dma_start(out=ky16[:], in_=_ap(stf, 0, [[1, 16], [16, 1]]))
    nc.vector.scalar_tensor_tensor(kx16[:], ky16[:], -4.0, idx16[:], ALU.mult, ALU.add)

    ones116f = persist.tile([1, 16], f32)
    nc.vector.memset(ones116f, 1.0)
    ones116 = persist.tile([1, 16], f32r)
    nc.scalar.activation(ones116[:], ones116f[:], AF.Copy, bias=0.0, scale=1.0)

    # B18L[:, li, 0:9]  = by16 - ky16 + fc_b[li, 0:9]   (hat arg offset, y)
    # B18L[:, li, 9:18] = bx16 - kx16 + fc_b[li, 9:18]  (hat arg offset, x)
    # folds the fc bias add and the base-coordinate shifts into one table.
    b18 = small.tile([16, 18], f32, tag="b18")
    nc.vector.tensor_scalar(b18[:, 0:9], by16[:], ky16[:], None, ALU.subtract)
    nc.vector.tensor_scalar(b18[:, 9:18], bx16[:], kx16[:], None, ALU.subtract)
    fcbr = persist.tile([1, NL * 18], f32r)
    nc.gpsimd.dma_start(out=fcbr[:], in_=_ap(fcb_d.bitcast(f32r), 0, [[0, 1], [1, NL * 18]]))
    fcb16_ps = psmall.tile([16, NL * 18], f32, tag="sp")
    nc.tensor.matmul(fcb16_ps[:], ones116[:], fcbr[:], start=True, stop=True)
    B18L = persist.tile([16, NL, 18], f32)
    for li in range(NL):
        nc.vector.tensor_tensor(B18L[:, li, :],
                                fcb16_ps[:, 18 * li:18 * li + 18], b18[:], ALU.add)

    kintf = persist.tile([16, 10], f32r)
    nc.vector.tensor_scalar(kintf[:], _ap(b18, 0, [b18.ap[0], [1, 10]]), 0.0,
                            None, ALU.mult)

    # fused conv2/conv3: W2fT[ci, t*4+j] = sum_c conv2_w[4c+j, ci, t] * conv3_w[c]
    w3sb = persist.tile([C, 4], f32r)
    nc.gpsimd.dma_start(out=w3sb[:].unsqueeze(-1), in_=_ap(w3_d.bitcast(f32r), 0, [[1, C], [0, 4], [1, 1]]))
    c2wj = persist.tile([C, 4, 576], f32r)
    for j in range(4):
        nc.gpsimd.dma_start(out=c2wj[:, j, :],
                            in_=_ap(w2_d.bitcast(f32r), j * 576, [[4 * 576, C], [1, 576]]))
    w2f_ps = psmall.tile([C, 144], f32, tag="sp")
    for t in range(9):
        for j in range(4):
            lhsT = _ap(c2wj, j * 576 + t, [c2wj.ap[0], [9, C]])
            k = (t * 4 + j) * 4
            nc.tensor.matmul(w2f_ps[:, k:k + 4], lhsT, w3sb[:],
                             start=True, stop=True)
    # block-diagonal fused conv2' weights: [128, 9, 8], M=8 (4 j x 2 halves)
    w2fbd = persist.tile([128, 9, 8], bf16)
    nc.gpsimd.memset(w2fbd[:], 0.0)
    nc.scalar.activation(w2fbd[0:C, :, 0:4],
                         _ap(w2f_ps, 0, [w2f_ps.ap[0], [16, 9], [4, 4]]),
                         AF.Copy, bias=0.0, scale=1.0)
    nc.gpsimd.dma_start(out=w2fbd[C:128, :, 4:8], in_=w2fbd[0:C, :, 0:4])
    c2bj = persist.tile([C, 4], f32r)
    nc.gpsimd.dma_start(out=c2bj[:], in_=_ap(b2_d.bitcast(f32r), 0, [[4, C], [1, 4]]))
    b2f_ps = psmall.tile([4, 4], f32, tag="sp")
    nc.tensor.matmul(b2f_ps[:], c2bj[:], w3sb[:], start=True, stop=True)
    b3b = small.tile([4, 1], f32, tag="b3b")
    nc.gpsimd.dma_start(out=b3b[:], in_=_ap(b3_d, 0, [[0, 4], [1, 1]]))
    b2f8 = persist.tile([8, 1], f32)
    nc.scalar.activation(b2f8[0:4, :], b2f_ps[:, 0:1], AF.Copy, bias=0.0, scale=1.0)
    nc.vector.tensor_add(b2f8[0:4, :], b2f8[0:4, :], b3b[:])
    nc.gpsimd.dma_start(out=b2f8[4:8, :], in_=b2f8[0:4, :])

    # selector for att replication: sel4[q, j, p] = 1 iff q == 4*(p>=64) + j
    sel4f = persist.tile([8, 4, 128], f32)
    sel4 = persist.tile([8, 4, 128], bf16)
    nc.gpsimd.memset(sel4f[:], 0.0)
    for g in range(2):
        # fill 1.0 where q - j - 4g == 0 over [8, 4, 64] view
        nc.gpsimd.affine_select(out=sel4f[:, :, C * g:C * g + C],
                                in_=sel4f[:, :, C * g:C * g + C],
                                pattern=[[-1, 4], [0, C]],
                                compare_op=ALU.not_equal,
                                fill=1.0, base=-4 * g, channel_multiplier=1)
    nc.scalar.activation(sel4[:], sel4f[:], AF.Copy, bias=0.0, scale=1.0)

    halo_fix(h1)

    # ---------------- dkc layers ----------------
    h_cur, pp_cur = h1, pp1
    for li in range(NL):
      with nc.named_scope(f"dkc{li}"):
        red = small.tile([128, 1], f32, tag="red")
        nc.vector.tensor_reduce(out=red[:], in_=pp_cur[:], axis=AX.X, op=ALU.add)
        # per-partition pool sum replicated to 16 columns -> one matmul
        # against precomputed G gives the 16-partition-replicated offsets
        nc.vector.tensor_scalar(red16[:], ones12816[:], red[:, 0:1],
                                None, ALU.mult)
        offp16 = psmall.tile([16, 18], f32, tag="sp")
        nc.tensor.matmul(offp16[:], red16[:], G[:, li, :],
                         start=True, stop=True)
        # hat function relu(1-|a|) = max(0, min(1-a, 1+a)), all on DVE;
        # a = off + (base - scope_coord) with fc bias folded into B18L
        a18 = small.tile([16, 18], f32, tag="a18")
        u18 = small.tile([16, 18], f32, tag="u18")
        nc.vector.tensor_tensor(a18[:], offp16[:], B18L[:, li, :], ALU.add)
        nc.vector.tensor_scalar(u18[:], a18[:], -1.0, 1.0, ALU.mult, ALU.add)
        nc.vector.tensor_scalar(a18[:], a18[:], 1.0, None, ALU.add)
        nc.vector.tensor_tensor(u18[:], u18[:], a18[:], ALU.min)
        nc.vector.tensor_scalar(u18[:], u18[:], 0.0, None, ALU.max)
        nc.vector.tensor_tensor(kintf[:, 0:9], u18[:, 0:9], u18[:, 9:18],
                                ALU.mult)
        samp_ps = psmall.tile([128, 10], f32, tag="sp")
        nc.tensor.matmul(samp_ps[:], w2dt2[:, li, :], kintf[:],
                         start=True, stop=True)
        # per-tap diag weights: first tap's build unblocks the matmul
        # stream while later taps still build (DVE reads samp psum direct)
        diag = {}
        for t in PE_TAPS:
            dt_ = diagp.tile([128, 128], bf16, tag="diag", name=f"diag_{li}_{t}")
            nc.vector.tensor_scalar(dt_[:], i128f[:], samp_ps[:, t:t + 1],
                                    None, ALU.mult)
            diag[t] = dt_

        h_nxt = new_h()
        if li < NL - 1:
            pp_nxt = small.tile([128, 16], f32, tag="pp")
        else:
            pp_nxt = None

        def win(b, t):
            s0 = 1 + 4 * b
            dy, dx = TAPS[t]
            return h_cur[:, s0 + dy:s0 + dy + 4, 1 + dx:1 + dx + HH]

        # tap-major waves of 2 blocks: each diag lhsT loads once per wave and
        # serves two back-to-back matmuls. Halo-dependent blocks 0/15 go last.
        order = list(range(1, 15)) + [0, 15]
        for w in range(8):
            blocks = order[2 * w:2 * w + 2]
            # waves 2+: DVE writes its taps into psum FIRST (overlapping the
            # previous waves' matmuls), then the PE taps accumulate on top
            # (start=False). The layer tail is then just the last ACT, not
            # a serial matmul->DVE->ACT chain. Waves 0-1 keep DVE-after so
            # the pre-writes don't sit on the critical path behind samp.
            dve_first = w >= 2
            pss = [psum.tile([128, 4, HH], f32, tag="cps", name=f"cps_{li}_{w}_{k}")
                   for k in range(len(blocks))]
            if dve_first:
                for k, b in enumerate(blocks):
                    t0, t1 = DVE_TAPS
                    nc.vector.tensor_scalar(pss[k][:], win(b, t0),
                                            samp_ps[:, t0:t0 + 1],
                                            None, ALU.mult)
                    nc.vector.scalar_tensor_tensor(pss[k][:], win(b, t1),
                                                   samp_ps[:, t1:t1 + 1],
                                                   pss[k][:], ALU.mult, ALU.add)
            for ti, t in enumerate(PE_TAPS):
                for k, b in enumerate(blocks):
                    nc.tensor.matmul(pss[k][:], diag[t][:], win(b, t),
                                     start=(ti == 0 and not dve_first),
                                     stop=(ti == len(PE_TAPS) - 1),
                                     skip_group_check=True)
            for k, b in enumerate(blocks):
                if not dve_first:
                    for t in DVE_TAPS:
                        nc.vector.scalar_tensor_tensor(pss[k][:], win(b, t),
                                                       samp_ps[:, t:t + 1],
                                                       pss[k][:], ALU.mult, ALU.add)
                s0 = 1 + 4 * b
                if pp_nxt is not None:
                    nc.scalar.activation(h_nxt[:, s0:s0 + 4, 1:HH + 1], pss[k][:],
                                         AF.Relu, bias=biases[:, 1 + li:2 + li],
                                         scale=1.0, accum_out=pp_nxt[:, b:b + 1])
                else:
                    nc.scalar.activation(h_nxt[:, s0:s0 + 4, 1:HH + 1], pss[k][:],
                                         AF.Relu, bias=biases[:, 1 + li:2 + li],
                                         scale=1.0)
        halo_fix(h_nxt)
        h_cur, pp_cur = h_nxt, pp_nxt

    # ------- fused conv2' -> sigmoid att -> replicate -> out = x * att -----
    # 1-block software pipeline: att for block b computes while block b-1's
    # replicate/multiply/store drains.
    att = persist.tile([8, RB, HH], bf16)
    # blocks 2..13 only need dkc4's early waves (its block order is
    # [1..14, 0, 15]); halo-dependent blocks 0/15 go last.
    forder = list(range(2, 14)) + [1, 14, 0, 15]
    with nc.named_scope("final"):
        for i in range(17):
            if i < 16:
                b = forder[i]
                s0 = 1 + 4 * b
                ps = psum.tile([128, 4, HH], f32, tag="cps")
                for t in range(9):
                    dy, dx = TAPS[t]
                    rhs = h_cur[:, s0 + dy:s0 + dy + 4, 1 + dx:1 + dx + HH]
                    nc.tensor.matmul(ps[0:8, :, :], w2fbd[:, t, :], rhs,
                                     start=(t == 0), stop=(t == 8))
                nc.scalar.activation(att[:, 4 * b:4 * b + 4, :], ps[0:8, :, :],
                                     AF.Sigmoid, bias=b2f8[:], scale=1.0)
            if i >= 1:
                bb = forder[i - 1]
                ob = outp.tile([128, 8, H], bf16, tag="ob")
                for j in range(4):
                    dy, dx = j // 2, j % 2
                    pa = pattn.tile([128, 4, HH], f32, tag="pa")
                    nc.tensor.matmul(pa[:], sel4[:, j, :],
                                     att[0:8, 4 * bb:4 * bb + 4, :],
                                     start=True, stop=True, skip_group_check=True)
                    xv = _ap(xsb, (8 * bb + 1 + dy) * H + dx,
                             [xsb.ap[0], [2 * H, 4], [2, HH]])
                    ov = _ap(ob, dy * H + dx, [ob.ap[0], [2 * H, 4], [2, HH]])
                    nc.vector.tensor_tensor(ov, xv, pa[:], ALU.mult)
                nc.sync.dma_start(out=o_d[:, 8 * bb:8 * bb + 8, :], in_=ob[0:C, :, :])
                nc.scalar.dma_start(out=o_d[:, 128 + 8 * bb:128 + 8 * bb + 8, :],
                                    in_=ob[C:128, :, :])


_NC_CACHE = {}

_W_NAMES = ["conv1_w", "conv1_b", "dkc_w", "dkc_b", "dkc_fc_w", "dkc_fc_b",
            "conv2_w", "conv2_b", "conv3_w", "conv3_b"]


def make_in_maps(inputs):
    shared = {n: np.ascontiguousarray(np.asarray(inputs[n], dtype=np.float32))
              for n in _W_NAMES}
    x = np.asarray(inputs["x"], dtype=np.float32).astype(ml_dtypes.bfloat16)
    return [dict(shared, x=np.ascontiguousarray(x[i])) for i in range(8)]


def kernel(**inputs):
    if "nc" not in _NC_CACHE:
        _NC_CACHE["nc"] = build_nc()
    nc = _NC_CACHE["nc"]
    in_maps = make_in_maps(inputs)
    r = run_bass_kernel_spmd(nc, in_maps, list(range(8)))
    _NC_CACHE["last_result"] = r
    return np.stack([r.results[i]["out"].astype(np.float32) for i in range(8)])
